# revision 2
# baseline (speedup 1.0000x reference)
"""Trainium2 Bass kernel for nn_CorrTrajBlock (sparse_attention) — v2.

Data-parallel over batch B=8 across 8 NeuronCores; one sample per core.

Per-core pipeline (C=512, T=8, H=W=28, HW=784, S=T*HW=6272, R=64, K=4,
Cq=128, P=T*R=512, E=640 ext-row):
  1. template_p = w_reduce_eff @ x[:, 0]    (fp32 matmul) ; argmax (DVE)
  2. tres via transpose-mode dma_gather from x_se (bf16, coords-extended),
     upcast to fp32 -> f32r for affinity lhsT
  3. per pi (2 frames): affinity (f32r matmul), top4 (DVE), index staging
     (1KB DRAM round trip + broadcast readback), ONE 512-row transpose
     gather -> gkp [c, 5cc, (k,p)] bf16; fuse matmul (incl. coords via
     extended channels), max over k (DVE), points accum (DVE)
  4. fmb=fm+b2 (bf16); conv over t (3 taps) + b3 + relu -> tc bf16;
     tcT via 16 PE transposes
  5. per s-tile: z = x^T @ pts (f32r, SBUF bitcast, no re-DMA);
     softmax WITHOUT max-subtract (exp scale=0.25, accum sum);
     proj transposes software-pipelined one iteration behind z
  6. prop = tcT^T @ proj (bf16 PSUM accum); +x via DVE add; one
     1MB output DMA per s-chunk
"""
import sys

sys.path.insert(0, "/opt/trn_rl_repo")

import numpy as np
import concourse.bass as bass
import concourse.mybir as mybir
import concourse.tile as tile
from concourse import bacc
from concourse.bass_utils import run_bass_kernel_spmd

F32 = mybir.dt.float32
F32R = mybir.dt.float32r
BF16 = mybir.dt.bfloat16
I16 = mybir.dt.int16
U32 = mybir.dt.uint32
AF = mybir.ActivationFunctionType
ALU = mybir.AluOpType
AX = mybir.AxisListType

B, C, T, H, W = 8, 512, 8, 28, 28
HW = H * W            # 784
S = T * HW            # 6272
R = 64
K = 4
Cq = 128
P = T * R             # 512
CC = C // 128         # 4
EC = 5                # extended channel chunks (4 data + 1 coords/pad)
E = EC * 128          # 640
NST = S // 128        # 49 s-tiles
NCH = 13              # s-chunks: 12 x 512 + 1 x 128

_CACHED = {}


def build_nc():
    nc = bacc.Bacc("TRN2", debug=False)

    X_CS = nc.dram_tensor("x_cs", [C, S], F32, kind="ExternalInput").ap()
    X_SC = nc.dram_tensor("x_sc", [S, C], F32, kind="ExternalInput").ap()
    X_SE = nc.dram_tensor("x_se", [S, E], BF16, kind="ExternalInput").ap()
    WRT = nc.dram_tensor("wrT", [C, R], F32, kind="ExternalInput").ap()
    WPE = nc.dram_tensor("wpe", [128, EC, Cq], BF16, kind="ExternalInput").ap()
    WTT = nc.dram_tensor("wtT", [3, Cq, C], BF16, kind="ExternalInput").ap()
    B2 = nc.dram_tensor("b2", [Cq, 1], F32, kind="ExternalInput").ap()
    B3 = nc.dram_tensor("b3", [CC, 128], F32, kind="ExternalInput").ap()
    IDB = nc.dram_tensor("identbf", [128, 128], BF16, kind="ExternalInput").ap()
    IDF = nc.dram_tensor("identf", [128, 128], F32, kind="ExternalInput").ap()
    OUT = nc.dram_tensor("out", [C, S], F32, kind="ExternalOutput").ap()

    L16 = nc.dram_tensor("l16", [128, 128], F32, kind="ExternalInput").ap()
    MEV = nc.dram_tensor("meven", [128, 1], F32, kind="ExternalInput").ap()
    MOD = nc.dram_tensor("modd", [128, 1], F32, kind="ExternalInput").ap()

    Xr = X_CS.rearrange("(cc p) s -> p cc s", p=128)
    OUTr = OUT.rearrange("(cc p) s -> p cc s", p=128)

    with tile.TileContext(nc) as tc:
        import contextlib
        ctx = contextlib.ExitStack()
        pers = ctx.enter_context(tc.tile_pool(name="pers", bufs=1))
        sb = ctx.enter_context(tc.tile_pool(name="sb", bufs=2))
        ps = ctx.enter_context(tc.tile_pool(name="ps", bufs=3, space="PSUM"))
        pstp = ctx.enter_context(tc.tile_pool(name="pstp", bufs=1, space="PSUM"))

        # ---- persistent loads ----
        idb_t = pers.tile([128, 128], BF16, tag="idb")
        nc.sync.dma_start(out=idb_t, in_=IDB)
        wrT_t = pers.tile([128, CC, R], F32, tag="wrT")
        nc.sync.dma_start(out=wrT_t, in_=WRT.rearrange("(cc p) r -> p cc r", p=128))
        xc = pers.tile([128, CC, S], F32R, tag="xc")
        Xrr = Xr.bitcast(F32R)
        nc.sync.dma_start(out=xc[:, :, 0:HW], in_=Xrr[:, :, 0:HW])  # frame 0 first
        # HAM warm-up: keep PE busy during the frame-0 DMA
        for _w in range(80):
            wu_ps = pstp.tile([128, 128], F32, tag="tp", name="wu_ps")
            nc.tensor.matmul(wu_ps, lhsT=idb_t, rhs=idb_t, start=True, stop=True)
        wpe_t = pers.tile([128, EC, Cq], BF16, tag="wpe")
        nc.sync.dma_start(out=wpe_t, in_=WPE)
        wtT_t = pers.tile([128, 3, C], BF16, tag="wtT")
        nc.sync.dma_start(out=wtT_t, in_=WTT.rearrange("d p c -> p d c"))
        b2_t = pers.tile([128, 1], F32, tag="b2")
        nc.sync.dma_start(out=b2_t, in_=B2)
        b3_t = pers.tile([128, CC], F32, tag="b3")
        nc.sync.dma_start(out=b3_t, in_=B3.rearrange("cc p -> p cc"))
        idf_t = pers.tile([128, 128], F32, tag="idf")
        nc.sync.dma_start(out=idf_t, in_=IDF)
        l16_t = pers.tile([128, 128], F32, tag="l16")
        nc.sync.dma_start(out=l16_t, in_=L16)
        mev_t = pers.tile([128, 1], F32, tag="mev")
        nc.sync.dma_start(out=mev_t, in_=MEV)
        mod_t = pers.tile([128, 1], F32, tag="mod")
        nc.sync.dma_start(out=mod_t, in_=MOD)
        for tb in range(1, T):
            nc.sync.dma_start(out=xc[:, :, tb * HW:(tb + 1) * HW],
                              in_=Xrr[:, :, tb * HW:(tb + 1) * HW])
        xcr = xc
        xcf = xc.bitcast(F32)

        # ---- phase 1: template (fp32, exact argmax) ----
        tpl_sb = pers.tile([64, HW], F32, tag="tpl")
        for h in range(2):
            tp_ps = ps.tile([64, 392], F32, tag="acc")
            for cc in range(CC):
                nc.tensor.matmul(tp_ps, lhsT=wrT_t[:, cc, :],
                                 rhs=xcf[:, cc, h * 392:(h + 1) * 392],
                                 start=(cc == 0), stop=(cc == CC - 1))
            nc.scalar.activation(tpl_sb[:, h * 392:(h + 1) * 392], tp_ps, AF.Copy)
        tmx = pers.tile([64, 8], F32, tag="tmx")
        tmi = pers.tile([64, 8], U32, tag="tmi")
        nc.vector.max(out=tmx, in_=tpl_sb)
        nc.vector.max_index(out=tmi, in_max=tmx, in_values=tpl_sb)
        spt_f = pers.tile([64, 1], F32, tag="spt_f")
        nc.vector.tensor_copy(spt_f, tmi[:, 0:1])
        gw0 = pers.tile([128, 8], F32, tag="gw0")
        nc.vector.memset(gw0, 0)
        sptA = pers.tile([64, 1], F32, tag="sptA")
        nc.vector.tensor_scalar(sptA, spt_f, mev_t[0:64, :], None, op0=ALU.mult)
        sptB = pers.tile([64, 1], F32, tag="sptB")
        nc.vector.tensor_scalar(sptB, spt_f, mod_t[0:64, :], None, op0=ALU.mult)
        for b in range(2):
            rows = slice(32 * b, 32 * (b + 1))
            nc.vector.tensor_copy(gw0[rows, 2 * b:2 * b + 1], sptA[rows, :])
            nc.vector.tensor_copy(gw0[rows, 2 * b + 1:2 * b + 2], sptB[rows, :])
        idx0ps = ps.tile([128, 8], F32, tag="acc")
        nc.tensor.matmul(idx0ps, lhsT=l16_t, rhs=gw0, start=True, stop=True)
        tidxt = pers.tile([128, 8], I16, tag="tidxt")
        nc.scalar.activation(tidxt, idx0ps, AF.Copy)
        tresT = pers.tile([128, 1, C], F32, tag="tresT")
        nc.gpsimd.dma_gather(out_ap=tresT, in_ap=X_SC, idxs_ap=tidxt,
                             num_idxs=128, num_idxs_reg=128, elem_size=C)
        for _w in range(36):
            wu_ps = pstp.tile([128, 128], F32, tag="tp", name="wu_ps")
            nc.tensor.matmul(wu_ps, lhsT=idb_t, rhs=idb_t, start=True, stop=True)
        tres_f = pers.tile([128, CC, R], F32, tag="tresf")
        for cc in range(CC):
            tpf = pstp.tile([128, 128], F32, tag="tp")
            nc.tensor.transpose(tpf, tresT[:, 0, cc * 128:(cc + 1) * 128], idf_t)
            nc.scalar.activation(tres_f[:, cc, :], tpf[:, 0:R], AF.Copy)

        # ---- phase 2+3 per pi: affinity, top4, gather, fuse, points ----
        fm_f32 = pers.tile([128, P], F32, tag="fmf")
        pts_f = pers.tile([128, CC, P], F32R, tag="ptsf")
        def fuse_points(pi, gkp):
            pcols = slice(128 * pi, 128 * (pi + 1))
            for k in range(K):
                f_ps = ps.tile([128, 128], F32, tag="acc")
                for cc in range(EC):
                    nc.tensor.matmul(f_ps, lhsT=wpe_t[:, cc, :],
                                     rhs=gkp[:, cc, 128 * k:128 * (k + 1)],
                                     start=(cc == 0), stop=(cc == EC - 1))
                if k == 0:
                    nc.scalar.activation(fm_f32[:, pcols], f_ps, AF.Copy)
                else:
                    nc.vector.tensor_tensor(out=fm_f32[:, pcols],
                                            in0=fm_f32[:, pcols], in1=f_ps,
                                            op=ALU.max)
            s0 = sb.tile([128, CC, 128], F32, tag="s0")
            nc.vector.tensor_tensor(out=s0, in0=gkp[:, 0:CC, 0:128],
                                    in1=gkp[:, 0:CC, 128:256], op=ALU.add)
            nc.vector.tensor_tensor(out=s0, in0=s0, in1=gkp[:, 0:CC, 256:384],
                                    op=ALU.add)
            nc.vector.tensor_tensor(out=pts_f[:, :, pcols], in0=s0,
                                    in1=gkp[:, 0:CC, 384:512], op=ALU.add)

        aff_tiles = {}
        ami_tiles = {}
        gkps = {}

        def aff_half(pi, h):
            if h == 0:
                aff_tiles[pi] = sb.tile([128, HW], F32, tag="aff",
                                        name="aff_sb")
            aff_sb = aff_tiles[pi]
            a_ps = ps.tile([128, 392], F32, tag="acc", name="a_ps")
            for tt in range(2):
                t = 2 * pi + tt
                for cc in range(CC):
                    nc.tensor.matmul(
                        a_ps[64 * tt:64 * (tt + 1), :],
                        lhsT=tres_f[:, cc, :],
                        rhs=xcf[:, cc, t * HW + h * 392: t * HW + (h + 1) * 392],
                        start=(cc == 0), stop=(cc == CC - 1),
                        tile_position=(0, 64 * tt))
            nc.scalar.activation(aff_sb[:, h * 392:(h + 1) * 392], a_ps, AF.Copy)
            if h == 1:
                amx = sb.tile([128, 8], F32, tag="amx", name="amx")
                ami = sb.tile([128, 8], U32, tag="ami", name="ami")
                nc.vector.max(out=amx, in_=aff_sb)
                nc.vector.max_index(out=ami, in_max=amx, in_values=aff_sb)
                ami_tiles[pi] = ami

        def staging(pi):
            ami = ami_tiles[pi]
            gw = sb.tile([128, 32], F32, tag="gw", name="gw")
            nc.vector.memset(gw, 0)
            gs2 = sb.tile([128, K], F32, tag="gs2", name="gs2")
            for tt in range(2):
                t = 2 * pi + tt
                rows = slice(64 * tt, 64 * (tt + 1))
                nc.vector.tensor_scalar(gs2[rows, :], ami[rows, 0:K],
                                        float(t * HW), None, op0=ALU.add)
            gsA = sb.tile([128, K], F32, tag="gsA", name="gsA")
            nc.vector.tensor_scalar(gsA, gs2, mev_t, None, op0=ALU.mult)
            gsB = sb.tile([128, K], F32, tag="gsB", name="gsB")
            nc.vector.tensor_scalar(gsB, gs2, mod_t, None, op0=ALU.mult)
            gwv = gw.rearrange("p (k c8) -> p k c8", c8=8)
            for b in range(4):
                rows = slice(32 * b, 32 * (b + 1))
                nc.vector.tensor_copy(gwv[rows, :, 2 * b], gsA[rows, :])
                nc.vector.tensor_copy(gwv[rows, :, 2 * b + 1], gsB[rows, :])
            idxps = ps.tile([128, 32], F32, tag="acc", name="idxps")
            nc.tensor.matmul(idxps, lhsT=l16_t, rhs=gw, start=True, stop=True)
            idx16 = sb.tile([128, 32], I16, tag="idx", name="idx16")
            nc.scalar.activation(idx16, idxps, AF.Copy)
            gkp = sb.tile([128, EC, 4 * 128], BF16, tag="gk", name="gkp")
            nc.gpsimd.dma_gather(out_ap=gkp, in_ap=X_SE, idxs_ap=idx16,
                                 num_idxs=512, num_idxs_reg=512, elem_size=E,
                                 transpose=True)
            gkps[pi] = gkp

        # schedule: A0_0 A1_0 A0_1 s0 A1_1 A0_2 f0 s1 A1_2 A0_3 f1 s2
        #           A1_3 f2 s3 f3  (PE never waits on the topk chain)
        aff_half(0, 0)
        aff_half(0, 1)
        aff_half(1, 0)
        staging(0)
        aff_half(1, 1)
        aff_half(2, 0)
        fuse_points(0, gkps[0])
        staging(1)
        aff_half(2, 1)
        aff_half(3, 0)
        fuse_points(1, gkps[1])
        staging(2)
        aff_half(3, 1)
        fuse_points(2, gkps[2])
        staging(3)
        fuse_points(3, gkps[3])
        pts_r = pts_f

        # ---- phase 3b: fm bias -> bf16; conv; tcT ----
        fmb = pers.tile([128, P], BF16, tag="fmb")
        nc.vector.tensor_scalar(fmb, fm_f32, b2_t, None, op0=ALU.add)
        tc_bf = pers.tile([128, CC, P], BF16, tag="tcbf")
        for ct in range(CC):
            c_ps = ps.tile([128, P], F32, tag="acc")
            cs = slice(ct * 128, (ct + 1) * 128)
            nc.tensor.matmul(c_ps, lhsT=wtT_t[:, 1, cs], rhs=fmb,
                             start=True, stop=False)
            nc.tensor.matmul(c_ps[:, R:P], lhsT=wtT_t[:, 0, cs], rhs=fmb[:, 0:P - R],
                             start=False, stop=False)
            nc.tensor.matmul(c_ps[:, 0:P - R], lhsT=wtT_t[:, 2, cs], rhs=fmb[:, R:P],
                             start=False, stop=True)
            nc.scalar.activation(tc_bf[:, ct, :], c_ps, AF.Relu,
                                 bias=b3_t[:, ct:ct + 1])
        tcT = pers.tile([128, CC, C], BF16, tag="tcT")
        for pb in range(4):
            tp2 = pstp.tile([128, C], BF16, tag="tp")
            for cc in range(CC):
                nc.tensor.transpose(tp2[:, cc * 128:(cc + 1) * 128],
                                    tc_bf[:, cc, pb * 128:(pb + 1) * 128], idb_t)
            nc.vector.tensor_copy(tcT[:, pb, :], tp2)

        # ---- phase 4: z -> softmax -> proj transpose (pipelined) -> prop ----
        projTP = {}
        proj_ch = {}
        pend = None  # (st, pjT tile) awaiting transposes

        def do_transposes(st_p, pjT_p):
            ch = st_p // 4
            sl = st_p % 4
            for pb in range(4):
                nc.tensor.transpose(projTP[ch][pb][:, sl * 128:(sl + 1) * 128],
                                    pjT_p[:, pb * 128:(pb + 1) * 128], idb_t)

        def do_prop(ch):
            cw = min(P, S - ch * P)
            nsl = (cw + 127) // 128
            for pb in range(4):
                nc.scalar.activation(proj_ch[ch][:, pb, 0:cw],
                                     projTP[ch][pb][:, 0:cw], AF.Copy)
            osb = sb.tile([128, CC, P], F32, tag="osb")
            for ct in range(CC):
                p_ps = ps.tile([128, P], F32, tag="acc")
                for pb in range(4):
                    nc.tensor.matmul(p_ps[:, 0:cw],
                                     lhsT=tcT[:, pb, ct * 128:(ct + 1) * 128],
                                     rhs=proj_ch[ch][:, pb, 0:cw],
                                     start=(pb == 0), stop=(pb == 3))
                nc.vector.tensor_tensor(out=osb[:, ct, 0:cw],
                                        in0=p_ps[:, 0:cw],
                                        in1=xcf[:, ct, ch * P:ch * P + cw],
                                        op=ALU.add)
            nc.sync.dma_start(out=OUTr[:, :, ch * P:ch * P + cw],
                              in_=osb[:, :, 0:cw])
            del projTP[ch], proj_ch[ch]

        for st in range(NST):
            chunk, slot = st // 4, st % 4
            if slot == 0:
                projTP[chunk] = [pstp.tile([128, P], BF16, tag=f"pj{i}",
                                           name=f"pj{i}")
                                 for i in range(4)]
                proj_ch[chunk] = sb.tile([128, 4, P], BF16, tag="projch",
                                         name="proj_ch")
            z_ps = ps.tile([128, P], F32, tag="acc")
            for cc in range(CC):
                nc.tensor.matmul(z_ps, lhsT=xcr[:, cc, st * 128:(st + 1) * 128],
                                 rhs=pts_r[:, cc, :],
                                 start=(cc == 0), stop=(cc == CC - 1))
            nm = sb.tile([128, 1], F32, tag="nm")
            nc.vector.tensor_reduce(nm, z_ps, axis=AX.X, op=ALU.max, negate=True)
            nm4 = sb.tile([128, 1], F32, tag="nm4")
            nc.vector.tensor_scalar(nm4, nm, 0.25, None, op0=ALU.mult)
            e_sb = sb.tile([128, P], F32, tag="esb")
            dsum = sb.tile([128, 1], F32, tag="dsum")
            nc.scalar.activation(e_sb, z_ps, AF.Exp, bias=nm4, scale=0.25,
                                 accum_out=dsum)
            rd = sb.tile([128, 1], F32, tag="rd")
            nc.vector.reciprocal(rd, dsum)
            pjT = sb.tile([128, P], BF16, tag="pjT")
            nc.vector.tensor_scalar(pjT, e_sb, rd, None, op0=ALU.mult)
            if pend is not None:
                do_transposes(*pend)
                if pend[0] % 4 == 3:
                    do_prop(pend[0] // 4)
            pend = (st, pjT)
        do_transposes(*pend)
        do_prop(pend[0] // 4)
        ctx.close()
    nc.compile()
    return nc


def _host_prep(inputs):
    eps = 1e-5
    f32 = np.float32
    import ml_dtypes
    bf16 = ml_dtypes.bfloat16
    x = np.asarray(inputs["input"], f32)                       # (B,C,T,H,W)
    s1 = np.asarray(inputs["bn1_gamma"]) / np.sqrt(np.asarray(inputs["bn1_var"]) + eps)
    wrT = (np.asarray(inputs["w_reduce"], f32) * s1[:, None]).T.astype(f32)
    s2 = np.asarray(inputs["bn2_gamma"]) / np.sqrt(np.asarray(inputs["bn2_var"]) + eps)
    wp = np.asarray(inputs["w_proj"], f32) * s2[:, None]       # (Cq, C+2)
    b2 = (np.asarray(inputs["bn2_beta"])
          - np.asarray(inputs["bn2_mean"]) * s2).astype(f32)
    s3 = np.asarray(inputs["bn3_gamma"]) / np.sqrt(np.asarray(inputs["bn3_var"]) + eps)
    wt = np.asarray(inputs["w_t"], f32)[:, :, :, 0] * s3[:, None, None]  # (C,Cq,3)
    b3 = (np.asarray(inputs["bn3_beta"])
          - np.asarray(inputs["bn3_mean"]) * s3).astype(f32)

    wpe = np.zeros((E, Cq), f32)
    wpe[:C] = wp[:, :C].T
    wpe[C] = wp[:, C]
    wpe[C + 1] = wp[:, C + 1]
    wpe_r = np.ascontiguousarray(
        wpe.reshape(EC, 128, Cq).transpose(1, 0, 2)).astype(bf16)

    common = {
        "wrT": np.ascontiguousarray(wrT),
        "wpe": wpe_r,
        "wtT": np.ascontiguousarray(np.transpose(wt, (2, 1, 0)).astype(bf16)),
        "b2": b2.reshape(Cq, 1),
        "b3": b3.reshape(CC, 128),
        "identbf": np.eye(128, dtype=bf16),
        "identf": np.eye(128, dtype=f32),
        "l16": np.tile(np.eye(16, dtype=f32), (8, 8)),
        "meven": (((np.arange(128) // 16) % 2 == 0)
                  .astype(f32).reshape(128, 1)),
        "modd": (((np.arange(128) // 16) % 2 == 1)
                 .astype(f32).reshape(128, 1)),
    }
    x_cs = x.reshape(B, C, S)
    x_sc = np.transpose(x_cs, (0, 2, 1))                       # (B,S,C)
    hw = (np.arange(S) % HW)
    rowc = ((hw // W).astype(f32) / H)
    colc = ((hw % W).astype(f32) / W)
    in_maps = []
    for b in range(B):
        m = dict(common)
        m["x_cs"] = np.ascontiguousarray(x_cs[b])
        m["x_sc"] = np.ascontiguousarray(x_sc[b])
        xse = np.zeros((S, E), bf16)
        xse[:, :C] = x_sc[b].astype(bf16)
        xse[:, C] = rowc.astype(bf16)
        xse[:, C + 1] = colc.astype(bf16)
        m["x_se"] = xse
        in_maps.append(m)
    return in_maps


def kernel(**inputs) -> np.ndarray:
    if "nc" not in _CACHED:
        _CACHED["nc"] = build_nc()
    nc = _CACHED["nc"]
    in_maps = _host_prep(inputs)
    res = run_bass_kernel_spmd(nc, in_maps, list(range(B)))
    out = np.stack([res.results[b]["out"] for b in range(B)], axis=0)
    return out.reshape(B, C, T, H, W).astype(np.float32)


# revision 3
# speedup vs baseline: 1.2131x; 1.2131x over previous
"""Trainium2 Bass kernel for nn_CorrTrajBlock (sparse_attention).

Data-parallel over batch B=8 across 8 NeuronCores; one sample per core.

Per-core pipeline (C=512, T=8, H=W=28, HW=784, S=T*HW=6272, R=64, K=4,
Cq=128, P=T*R=512, E=640 coords-extended row):
  1. x resident in SBUF as f32r [c, cc, s] (single 12.8MB load, bitcast
     to fp32 where exactness is needed); PE HAM warm-up matmuls overlap
     the initial DMA.
  2. template_p = w_reduce_eff @ x[:,0] (fp32 matmul, exact argmax on
     DVE).  Gather indices are staged ON-CHIP: masked diagonal layout +
     one L16 pattern matmul (L16[p,o] = p%16==o%16) simultaneously
     folds partitions to the [16, n] wrap dma_gather wants AND
     replicates across the 8 gpsimd cores.  No DRAM round trips.
  3. tres = 128-row fp32 dma_gather from x_sc + 4 PE transposes
     (affinity MUST be exact fp32: top-4 margins are ~1e-5 relative;
     bf16/f32r affinity flips topk picks and fails vs the reference).
  4. Software-pipelined per-pi (2 frames) schedule
     A0 A1 | s | f ... interleaving fp32 affinity halves, on-chip index
     staging, ONE 512-row transpose-mode bf16 gather per pi from x_se
     (coords precomputed as 2 extra channels -> fuse needs no separate
     coords matmul), fuse (bf16) + max over k, points accum (DVE,
     written as f32r for the z matmul).
  5. fm+b2 -> bf16; conv over t (3 taps, column-shifted matmuls) + b3
     + relu; tcT via 16 PE transposes.
  6. Per s-tile (49): z = x^T @ pts (f32r, N=512), row-max (DVE), exp
     (ACT, scale=0.25 folds the mean-over-K), reciprocal+scale (DVE)
     -> proj transposes software-pipelined one tile behind z so the PE
     never stalls on the softmax chain; per chunk of 4 tiles: prop =
     tcT^T @ proj (bf16 PSUM accum), +x via DVE add, one 1MB output
     DMA.
"""
import sys

sys.path.insert(0, "/opt/trn_rl_repo")

import numpy as np
import concourse.bass as bass
import concourse.mybir as mybir
import concourse.tile as tile
from concourse import bacc
from concourse.bass_utils import run_bass_kernel_spmd

F32 = mybir.dt.float32
F32R = mybir.dt.float32r
BF16 = mybir.dt.bfloat16
I16 = mybir.dt.int16
U32 = mybir.dt.uint32
AF = mybir.ActivationFunctionType
ALU = mybir.AluOpType
AX = mybir.AxisListType

B, C, T, H, W = 8, 512, 8, 28, 28
HW = H * W            # 784
S = T * HW            # 6272
R = 64
K = 4
Cq = 128
P = T * R             # 512
CC = C // 128         # 4
EC = 5                # extended channel chunks (4 data + 1 coords/pad)
E = EC * 128          # 640
NST = S // 128        # 49 s-tiles
NCH = 13              # s-chunks: 12 x 512 + 1 x 128

_CACHED = {}


def build_nc():
    nc = bacc.Bacc("TRN2", debug=False)

    X_CS = nc.dram_tensor("x_cs", [C, S], F32, kind="ExternalInput").ap()
    X_SC = nc.dram_tensor("x_sc", [S, C], F32, kind="ExternalInput").ap()
    X_SE = nc.dram_tensor("x_se", [S, E], BF16, kind="ExternalInput").ap()
    WRT = nc.dram_tensor("wrT", [C, R], F32, kind="ExternalInput").ap()
    WPE = nc.dram_tensor("wpe", [128, EC, Cq], BF16, kind="ExternalInput").ap()
    WTT = nc.dram_tensor("wtT", [3, Cq, C], BF16, kind="ExternalInput").ap()
    B2 = nc.dram_tensor("b2", [Cq, 1], F32, kind="ExternalInput").ap()
    B3 = nc.dram_tensor("b3", [CC, 128], F32, kind="ExternalInput").ap()
    IDB = nc.dram_tensor("identbf", [128, 128], BF16, kind="ExternalInput").ap()
    IDF = nc.dram_tensor("identf", [128, 128], F32, kind="ExternalInput").ap()
    OUT = nc.dram_tensor("out", [C, S], F32, kind="ExternalOutput").ap()

    L16 = nc.dram_tensor("l16", [128, 128], F32, kind="ExternalInput").ap()
    MEV = nc.dram_tensor("meven", [128, 1], F32, kind="ExternalInput").ap()
    MOD = nc.dram_tensor("modd", [128, 1], F32, kind="ExternalInput").ap()

    Xr = X_CS.rearrange("(cc p) s -> p cc s", p=128)
    OUTr = OUT.rearrange("(cc p) s -> p cc s", p=128)

    with tile.TileContext(nc) as tc:
        import contextlib
        ctx = contextlib.ExitStack()
        pers = ctx.enter_context(tc.tile_pool(name="pers", bufs=1))
        sb = ctx.enter_context(tc.tile_pool(name="sb", bufs=2))
        ps = ctx.enter_context(tc.tile_pool(name="ps", bufs=3, space="PSUM"))
        pstp = ctx.enter_context(tc.tile_pool(name="pstp", bufs=1, space="PSUM"))

        # ---- persistent loads ----
        idb_t = pers.tile([128, 128], BF16, tag="idb")
        nc.sync.dma_start(out=idb_t, in_=IDB)
        wrT_t = pers.tile([128, CC, R], F32, tag="wrT")
        nc.sync.dma_start(out=wrT_t, in_=WRT.rearrange("(cc p) r -> p cc r", p=128))
        xc = pers.tile([128, CC, S], F32R, tag="xc")
        Xrr = Xr.bitcast(F32R)
        nc.sync.dma_start(out=xc[:, :, 0:HW], in_=Xrr[:, :, 0:HW])  # frame 0 first
        # HAM warm-up: keep PE busy during the frame-0 DMA
        for _w in range(12):
            wu_ps = pstp.tile([128, 128], F32, tag="tp", name="wu_ps")
            nc.tensor.matmul(wu_ps, lhsT=idb_t, rhs=idb_t, start=True, stop=True)
        wpe_t = pers.tile([128, EC, Cq], BF16, tag="wpe")
        nc.sync.dma_start(out=wpe_t, in_=WPE)
        wtT_t = pers.tile([128, 3, C], BF16, tag="wtT")
        nc.sync.dma_start(out=wtT_t, in_=WTT.rearrange("d p c -> p d c"))
        b2_t = pers.tile([128, 1], F32, tag="b2")
        nc.sync.dma_start(out=b2_t, in_=B2)
        b3_t = pers.tile([128, CC], F32, tag="b3")
        nc.sync.dma_start(out=b3_t, in_=B3.rearrange("cc p -> p cc"))
        idf_t = pers.tile([128, 128], F32, tag="idf")
        nc.sync.dma_start(out=idf_t, in_=IDF)
        l16_t = pers.tile([128, 128], F32, tag="l16")
        nc.sync.dma_start(out=l16_t, in_=L16)
        mev_t = pers.tile([128, 1], F32, tag="mev")
        nc.sync.dma_start(out=mev_t, in_=MEV)
        mod_t = pers.tile([128, 1], F32, tag="mod")
        nc.sync.dma_start(out=mod_t, in_=MOD)
        for tb in range(1, T):
            nc.sync.dma_start(out=xc[:, :, tb * HW:(tb + 1) * HW],
                              in_=Xrr[:, :, tb * HW:(tb + 1) * HW])
        xcr = xc
        xcf = xc.bitcast(F32)

        # ---- phase 1: template (fp32, exact argmax) ----
        tpl_sb = pers.tile([64, HW], F32, tag="tpl")
        for h in range(2):
            tp_ps = ps.tile([64, 392], F32, tag="acc")
            for cc in range(CC):
                nc.tensor.matmul(tp_ps, lhsT=wrT_t[:, cc, :],
                                 rhs=xcf[:, cc, h * 392:(h + 1) * 392],
                                 start=(cc == 0), stop=(cc == CC - 1))
            nc.scalar.activation(tpl_sb[:, h * 392:(h + 1) * 392], tp_ps, AF.Copy)
        tmx = pers.tile([64, 8], F32, tag="tmx")
        tmi = pers.tile([64, 8], U32, tag="tmi")
        nc.vector.max(out=tmx, in_=tpl_sb)
        nc.vector.max_index(out=tmi, in_max=tmx, in_values=tpl_sb)
        spt_f = pers.tile([64, 1], F32, tag="spt_f")
        nc.vector.tensor_copy(spt_f, tmi[:, 0:1])
        gw0 = pers.tile([128, 8], F32, tag="gw0")
        nc.vector.memset(gw0, 0)
        sptA = pers.tile([64, 1], F32, tag="sptA")
        nc.vector.tensor_scalar(sptA, spt_f, mev_t[0:64, :], None, op0=ALU.mult)
        sptB = pers.tile([64, 1], F32, tag="sptB")
        nc.vector.tensor_scalar(sptB, spt_f, mod_t[0:64, :], None, op0=ALU.mult)
        for b in range(2):
            rows = slice(32 * b, 32 * (b + 1))
            nc.vector.tensor_copy(gw0[rows, 2 * b:2 * b + 1], sptA[rows, :])
            nc.vector.tensor_copy(gw0[rows, 2 * b + 1:2 * b + 2], sptB[rows, :])
        idx0ps = ps.tile([128, 8], F32, tag="acc")
        nc.tensor.matmul(idx0ps, lhsT=l16_t, rhs=gw0, start=True, stop=True)
        tidxt = pers.tile([128, 8], I16, tag="tidxt")
        nc.scalar.activation(tidxt, idx0ps, AF.Copy)
        tresT = pers.tile([128, 1, C], F32, tag="tresT")
        nc.gpsimd.dma_gather(out_ap=tresT, in_ap=X_SC, idxs_ap=tidxt,
                             num_idxs=128, num_idxs_reg=128, elem_size=C)
        for _w in range(10):
            wu_ps = pstp.tile([128, 128], F32, tag="tp", name="wu_ps")
            nc.tensor.matmul(wu_ps, lhsT=idb_t, rhs=idb_t, start=True, stop=True)
        tres_f = pers.tile([128, CC, R], F32, tag="tresf")
        for cc in range(CC):
            tpf = pstp.tile([128, 128], F32, tag="tp")
            nc.tensor.transpose(tpf, tresT[:, 0, cc * 128:(cc + 1) * 128], idf_t)
            nc.scalar.activation(tres_f[:, cc, :], tpf[:, 0:R], AF.Copy)

        # ---- phase 2+3 per pi: affinity, top4, gather, fuse, points ----
        fm_f32 = pers.tile([128, P], F32, tag="fmf")
        pts_f = pers.tile([128, CC, P], F32R, tag="ptsf")
        def fuse_points(pi, gkp):
            pcols = slice(128 * pi, 128 * (pi + 1))
            for k in range(K):
                f_ps = ps.tile([128, 128], F32, tag="acc")
                for cc in range(EC):
                    nc.tensor.matmul(f_ps, lhsT=wpe_t[:, cc, :],
                                     rhs=gkp[:, cc, 128 * k:128 * (k + 1)],
                                     start=(cc == 0), stop=(cc == EC - 1))
                if k == 0:
                    nc.scalar.activation(fm_f32[:, pcols], f_ps, AF.Copy)
                else:
                    nc.vector.tensor_tensor(out=fm_f32[:, pcols],
                                            in0=fm_f32[:, pcols], in1=f_ps,
                                            op=ALU.max)
            s0 = sb.tile([128, CC, 128], F32, tag="s0")
            nc.vector.tensor_tensor(out=s0, in0=gkp[:, 0:CC, 0:128],
                                    in1=gkp[:, 0:CC, 128:256], op=ALU.add)
            nc.vector.tensor_tensor(out=s0, in0=s0, in1=gkp[:, 0:CC, 256:384],
                                    op=ALU.add)
            nc.vector.tensor_tensor(out=pts_f[:, :, pcols], in0=s0,
                                    in1=gkp[:, 0:CC, 384:512], op=ALU.add)

        aff_tiles = {}
        ami_tiles = {}
        gkps = {}

        def aff_half(pi, h):
            if h == 0:
                aff_tiles[pi] = sb.tile([128, HW], F32, tag="aff",
                                        name="aff_sb")
            aff_sb = aff_tiles[pi]
            a_ps = ps.tile([128, 392], F32, tag="acc", name="a_ps")
            for tt in range(2):
                t = 2 * pi + tt
                for cc in range(CC):
                    nc.tensor.matmul(
                        a_ps[64 * tt:64 * (tt + 1), :],
                        lhsT=tres_f[:, cc, :],
                        rhs=xcf[:, cc, t * HW + h * 392: t * HW + (h + 1) * 392],
                        start=(cc == 0), stop=(cc == CC - 1),
                        tile_position=(0, 64 * tt))
            nc.scalar.activation(aff_sb[:, h * 392:(h + 1) * 392], a_ps, AF.Copy)
            if h == 1:
                amx = sb.tile([128, 8], F32, tag="amx", name="amx")
                ami = sb.tile([128, 8], U32, tag="ami", name="ami")
                nc.vector.max(out=amx, in_=aff_sb)
                nc.vector.max_index(out=ami, in_max=amx, in_values=aff_sb)
                ami_tiles[pi] = ami

        def staging(pi):
            ami = ami_tiles[pi]
            gw = sb.tile([128, 32], F32, tag="gw", name="gw")
            nc.vector.memset(gw, 0)
            gs2 = sb.tile([128, K], F32, tag="gs2", name="gs2")
            for tt in range(2):
                t = 2 * pi + tt
                rows = slice(64 * tt, 64 * (tt + 1))
                nc.vector.tensor_scalar(gs2[rows, :], ami[rows, 0:K],
                                        float(t * HW), None, op0=ALU.add)
            gsA = sb.tile([128, K], F32, tag="gsA", name="gsA")
            nc.vector.tensor_scalar(gsA, gs2, mev_t, None, op0=ALU.mult)
            gsB = sb.tile([128, K], F32, tag="gsB", name="gsB")
            nc.vector.tensor_scalar(gsB, gs2, mod_t, None, op0=ALU.mult)
            gwv = gw.rearrange("p (k c8) -> p k c8", c8=8)
            for b in range(4):
                rows = slice(32 * b, 32 * (b + 1))
                nc.vector.tensor_copy(gwv[rows, :, 2 * b], gsA[rows, :])
                nc.vector.tensor_copy(gwv[rows, :, 2 * b + 1], gsB[rows, :])
            idxps = ps.tile([128, 32], F32, tag="acc", name="idxps")
            nc.tensor.matmul(idxps, lhsT=l16_t, rhs=gw, start=True, stop=True)
            idx16 = sb.tile([128, 32], I16, tag="idx", name="idx16")
            nc.scalar.activation(idx16, idxps, AF.Copy)
            gkp = sb.tile([128, EC, 4 * 128], BF16, tag="gk", name="gkp")
            nc.gpsimd.dma_gather(out_ap=gkp, in_ap=X_SE, idxs_ap=idx16,
                                 num_idxs=512, num_idxs_reg=512, elem_size=E,
                                 transpose=True)
            gkps[pi] = gkp

        # schedule: A0_0 A1_0 A0_1 s0 A1_1 A0_2 f0 s1 A1_2 A0_3 f1 s2
        #           A1_3 f2 s3 f3  (PE never waits on the topk chain)
        aff_half(0, 0)
        aff_half(0, 1)
        aff_half(1, 0)
        staging(0)
        aff_half(1, 1)
        aff_half(2, 0)
        fuse_points(0, gkps[0])
        staging(1)
        aff_half(2, 1)
        aff_half(3, 0)
        fuse_points(1, gkps[1])
        staging(2)
        aff_half(3, 1)
        fuse_points(2, gkps[2])
        staging(3)
        fuse_points(3, gkps[3])
        pts_r = pts_f

        # ---- phase 3b: fm bias -> bf16; conv; tcT ----
        fmb = pers.tile([128, P], BF16, tag="fmb")
        nc.vector.tensor_scalar(fmb, fm_f32, b2_t, None, op0=ALU.add)
        tc_bf = pers.tile([128, CC, P], BF16, tag="tcbf")
        for ct in range(CC):
            c_ps = ps.tile([128, P], F32, tag="acc")
            cs = slice(ct * 128, (ct + 1) * 128)
            nc.tensor.matmul(c_ps, lhsT=wtT_t[:, 1, cs], rhs=fmb,
                             start=True, stop=False)
            nc.tensor.matmul(c_ps[:, R:P], lhsT=wtT_t[:, 0, cs], rhs=fmb[:, 0:P - R],
                             start=False, stop=False)
            nc.tensor.matmul(c_ps[:, 0:P - R], lhsT=wtT_t[:, 2, cs], rhs=fmb[:, R:P],
                             start=False, stop=True)
            nc.scalar.activation(tc_bf[:, ct, :], c_ps, AF.Relu,
                                 bias=b3_t[:, ct:ct + 1])
        tcT = pers.tile([128, CC, C], BF16, tag="tcT")
        for pb in range(4):
            tp2 = pstp.tile([128, C], BF16, tag="tp")
            for cc in range(CC):
                nc.tensor.transpose(tp2[:, cc * 128:(cc + 1) * 128],
                                    tc_bf[:, cc, pb * 128:(pb + 1) * 128], idb_t)
            nc.vector.tensor_copy(tcT[:, pb, :], tp2)

        # ---- phase 4: z -> softmax -> proj transpose (pipelined) -> prop ----
        projTP = {}
        proj_ch = {}
        pend = None  # (st, pjT tile) awaiting transposes

        def do_transposes(st_p, pjT_p):
            ch = st_p // 4
            sl = st_p % 4
            for pb in range(4):
                nc.tensor.transpose(projTP[ch][pb][:, sl * 128:(sl + 1) * 128],
                                    pjT_p[:, pb * 128:(pb + 1) * 128], idb_t)

        def do_prop(ch):
            cw = min(P, S - ch * P)
            nsl = (cw + 127) // 128
            for pb in range(4):
                nc.scalar.activation(proj_ch[ch][:, pb, 0:cw],
                                     projTP[ch][pb][:, 0:cw], AF.Copy)
            osb = sb.tile([128, CC, P], F32, tag="osb")
            for ct in range(CC):
                p_ps = ps.tile([128, P], F32, tag="acc")
                for pb in range(4):
                    nc.tensor.matmul(p_ps[:, 0:cw],
                                     lhsT=tcT[:, pb, ct * 128:(ct + 1) * 128],
                                     rhs=proj_ch[ch][:, pb, 0:cw],
                                     start=(pb == 0), stop=(pb == 3))
                nc.vector.tensor_tensor(out=osb[:, ct, 0:cw],
                                        in0=p_ps[:, 0:cw],
                                        in1=xcf[:, ct, ch * P:ch * P + cw],
                                        op=ALU.add)
            nc.sync.dma_start(out=OUTr[:, :, ch * P:ch * P + cw],
                              in_=osb[:, :, 0:cw])
            del projTP[ch], proj_ch[ch]

        for st in range(NST):
            chunk, slot = st // 4, st % 4
            if slot == 0:
                projTP[chunk] = [pstp.tile([128, P], BF16, tag=f"pj{i}",
                                           name=f"pj{i}")
                                 for i in range(4)]
                proj_ch[chunk] = sb.tile([128, 4, P], BF16, tag="projch",
                                         name="proj_ch")
            z_ps = ps.tile([128, P], F32, tag="acc")
            for cc in range(CC):
                nc.tensor.matmul(z_ps, lhsT=xcr[:, cc, st * 128:(st + 1) * 128],
                                 rhs=pts_r[:, cc, :],
                                 start=(cc == 0), stop=(cc == CC - 1))
            nm = sb.tile([128, 1], F32, tag="nm")
            nc.vector.tensor_reduce(nm, z_ps, axis=AX.X, op=ALU.max, negate=True)
            nm4 = sb.tile([128, 1], F32, tag="nm4")
            nc.vector.tensor_scalar(nm4, nm, 0.25, None, op0=ALU.mult)
            e_sb = sb.tile([128, P], F32, tag="esb")
            dsum = sb.tile([128, 1], F32, tag="dsum")
            nc.scalar.activation(e_sb, z_ps, AF.Exp, bias=nm4, scale=0.25,
                                 accum_out=dsum)
            rd = sb.tile([128, 1], F32, tag="rd")
            nc.vector.reciprocal(rd, dsum)
            pjT = sb.tile([128, P], BF16, tag="pjT")
            nc.vector.tensor_scalar(pjT, e_sb, rd, None, op0=ALU.mult)
            if pend is not None:
                do_transposes(*pend)
                if pend[0] % 4 == 3:
                    do_prop(pend[0] // 4)
            pend = (st, pjT)
        do_transposes(*pend)
        do_prop(pend[0] // 4)
        ctx.close()
    nc.compile()
    return nc


def _host_prep(inputs):
    eps = 1e-5
    f32 = np.float32
    import ml_dtypes
    bf16 = ml_dtypes.bfloat16
    x = np.asarray(inputs["input"], f32)                       # (B,C,T,H,W)
    s1 = np.asarray(inputs["bn1_gamma"]) / np.sqrt(np.asarray(inputs["bn1_var"]) + eps)
    wrT = (np.asarray(inputs["w_reduce"], f32) * s1[:, None]).T.astype(f32)
    s2 = np.asarray(inputs["bn2_gamma"]) / np.sqrt(np.asarray(inputs["bn2_var"]) + eps)
    wp = np.asarray(inputs["w_proj"], f32) * s2[:, None]       # (Cq, C+2)
    b2 = (np.asarray(inputs["bn2_beta"])
          - np.asarray(inputs["bn2_mean"]) * s2).astype(f32)
    s3 = np.asarray(inputs["bn3_gamma"]) / np.sqrt(np.asarray(inputs["bn3_var"]) + eps)
    wt = np.asarray(inputs["w_t"], f32)[:, :, :, 0] * s3[:, None, None]  # (C,Cq,3)
    b3 = (np.asarray(inputs["bn3_beta"])
          - np.asarray(inputs["bn3_mean"]) * s3).astype(f32)

    wpe = np.zeros((E, Cq), f32)
    wpe[:C] = wp[:, :C].T
    wpe[C] = wp[:, C]
    wpe[C + 1] = wp[:, C + 1]
    wpe_r = np.ascontiguousarray(
        wpe.reshape(EC, 128, Cq).transpose(1, 0, 2)).astype(bf16)

    common = {
        "wrT": np.ascontiguousarray(wrT),
        "wpe": wpe_r,
        "wtT": np.ascontiguousarray(np.transpose(wt, (2, 1, 0)).astype(bf16)),
        "b2": b2.reshape(Cq, 1),
        "b3": b3.reshape(CC, 128),
        "identbf": np.eye(128, dtype=bf16),
        "identf": np.eye(128, dtype=f32),
        "l16": np.tile(np.eye(16, dtype=f32), (8, 8)),
        "meven": (((np.arange(128) // 16) % 2 == 0)
                  .astype(f32).reshape(128, 1)),
        "modd": (((np.arange(128) // 16) % 2 == 1)
                 .astype(f32).reshape(128, 1)),
    }
    x_cs = x.reshape(B, C, S)
    x_sc = np.transpose(x_cs, (0, 2, 1))                       # (B,S,C)
    hw = (np.arange(S) % HW)
    rowc = ((hw // W).astype(f32) / H)
    colc = ((hw % W).astype(f32) / W)
    in_maps = []
    for b in range(B):
        m = dict(common)
        m["x_cs"] = np.ascontiguousarray(x_cs[b])
        m["x_sc"] = np.ascontiguousarray(x_sc[b])
        xse = np.zeros((S, E), bf16)
        xse[:, :C] = x_sc[b].astype(bf16)
        xse[:, C] = rowc.astype(bf16)
        xse[:, C + 1] = colc.astype(bf16)
        m["x_se"] = xse
        in_maps.append(m)
    return in_maps


def kernel(**inputs) -> np.ndarray:
    if "nc" not in _CACHED:
        _CACHED["nc"] = build_nc()
    nc = _CACHED["nc"]
    in_maps = _host_prep(inputs)
    res = run_bass_kernel_spmd(nc, in_maps, list(range(B)))
    out = np.stack([res.results[b]["out"] for b in range(B)], axis=0)
    return out.reshape(B, C, T, H, W).astype(np.float32)
